# revision 1
# baseline (speedup 1.0000x reference)
"""Causal attention (B=4096, T=64, C=64) on 8 TRN2 NeuronCores, pure data parallel.

Per core: x shard [512, 64, 64]. 512-token macro-tiles (8 batches), bf16 matmul
operands (f32 PSUM accumulate), 2-way tile_position packing: even batches on
out partitions 0:64 (array cols 0:64), odd on 64:128 - only the proven-safe
position set {(0,0), (0,64), (64,64), full-K} is used (mixing array-row
sources into one psum partition range crashes the PE).

Per tile: x DMA'd permuted (partition p <- tokens 4p..4p+3, 1KB descriptors,
one DMA per 2 tiles); Pool casts to bf16; PE transposes bf16 to contiguous
(m p)-packed psum; DVE does a straight 2-byte copy (2x mode) to SBUF; Pool
un-permutes SBUF->SBUF into natural token order (rows 0:64 of a persistent
[65, 512] tile whose row 64 is constant ones).

Folds:
  A = Wq^T Wk: hT = A^T xT in one matmul on the packed xT; the PSUM->SBUF
    cast un-permutes to natural order (Act).
  B_aug65 [65, 65] = [Wv^T Wp^T, 0; bp, 1]: with the ones row of the xT tile
    (K=65), xB = x@B + bp and col 64 = softmax sums source - no bias matmul,
    no ones memset.
  (y + sums*bp) * recip = y*recip + bp: epilogue is one broadcast
    tensor_tensor multiply (DVE) + reciprocal.

Engine split (Pool cannot read PSUM): Pool = cast + un-permute; DVE = xT
copy + recip + scale + part of xB copy; Act = hT copy + exp + rest of xB
copy; y store on HWDGE from SP (SWDGE would cost Pool ~1.2us/tile).
Emission is software-pipelined (12 stages, 1 tile of skew each, reverse-lag
emission order) so no engine's in-order stream waits on the cross-engine
dependency chain; PSUM tags A/B/D/Y ring-buffer across iterations.
"""

import numpy as np
import ml_dtypes

import concourse.bass as bass
import concourse.mybir as mybir
import concourse.tile as tile
import concourse.masks as masks
from concourse import bacc

F32 = mybir.dt.float32
BF16 = mybir.dt.bfloat16

N_CORES = 8
B, T, C = 4096, 64, 64
B_LOC = B // N_CORES  # 512 batches per core

MASK_VAL = -1e9


def build_nc(b_loc=B_LOC, batches_per_tile=8, reps=1, n_stages_dbg=None, dup=()):
    """Build the single-core Bass graph (SPMD: same graph on all 8 cores)."""
    assert b_loc % batches_per_tile == 0
    n_tiles = b_loc // batches_per_tile
    TOK = batches_per_tile * T              # tokens per macro tile (512)
    NCH = TOK // 128                        # batch pairs per tile (4)
    NB = batches_per_tile                   # batches per tile (8)

    nc = bacc.Bacc("TRN2", target_bir_lowering=False, debug=False)

    x_ext = nc.declare_dram_parameter("x", [b_loc, T, C], F32, isOutput=False)
    Wk_ext = nc.declare_dram_parameter("Wk", [C, C], F32, isOutput=False)
    Wq_ext = nc.declare_dram_parameter("Wq", [C, C], F32, isOutput=False)
    Wv_ext = nc.declare_dram_parameter("Wv", [C, C], F32, isOutput=False)
    Wp_ext = nc.declare_dram_parameter("Wp", [C, C], F32, isOutput=False)
    bp_ext = nc.declare_dram_parameter("bp", [C], F32, isOutput=False)
    out_ext = nc.declare_dram_parameter("out", [b_loc, T, C], F32, isOutput=True)

    x_flat = x_ext.ap().rearrange("b t c -> (b t) c")
    out_flat = out_ext.ap().rearrange("b t c -> (b t) c")

    bf = ml_dtypes.bfloat16
    m1 = np.where(
        np.arange(T)[:, None] <= np.arange(T)[None, :], 0.0, MASK_VAL * 8.0
    ).astype(np.float32)
    maskS2_dram = nc.inline_tensor(
        np.vstack([m1, m1]).astype(bf), name="maskS2_const"
    )
    # one-hot rows for xT_aug (natural order): col t -> row (t % 64)
    identT8 = np.tile(np.eye(T, dtype=np.float32), (1, batches_per_tile))
    identT8_dram = nc.inline_tensor(identT8.astype(bf), name="identT8_const")
    ident_dram = nc.inline_tensor(np.eye(128, dtype=np.float32), name="ident_const")

    AUG_BUFS = 7

    with tile.TileContext(nc) as tc:
        with (
            tc.tile_pool(name="const", bufs=1) as constp,
            tc.tile_pool(name="xin", bufs=4) as xin_pool,
            tc.tile_pool(name="xp", bufs=3) as xp_pool,
            tc.tile_pool(name="hp", bufs=3) as hp_pool,
            tc.tile_pool(name="ht", bufs=4) as ht_pool,
            tc.tile_pool(name="wei", bufs=4) as wei_pool,
            tc.tile_pool(name="xb", bufs=3) as xb_pool,
            tc.tile_pool(name="rc", bufs=3) as rc_pool,
            tc.tile_pool(name="yout", bufs=4) as yout_pool,
            tc.tile_pool(name="ps", bufs=2, space="PSUM") as ps,
        ):
            # ---- one-time constants ----
            ident = constp.tile([128, 128], F32)
            nc.sync.dma_start(ident[:], ident_dram.ap())
            ident_bf = constp.tile([128, 128], BF16)
            masks.make_identity(nc, ident_bf[:])
            maskS2 = constp.tile([128, T], BF16)
            nc.sync.dma_start(maskS2[:], maskS2_dram.ap())

            # weights: natural DMA (contiguous), PE transpose, cast to bf16
            wnat = constp.tile([C, 4 * C], F32)
            for i, w_ext in enumerate((Wq_ext, Wk_ext, Wv_ext, Wp_ext)):
                nc.sync.dma_start(wnat[:, i * C : (i + 1) * C], w_ext.ap())
            wT_ps = ps.tile([C, 4 * C], F32, tag="A")
            for i in range(4):
                nc.tensor.transpose(
                    wT_ps[:, i * C : (i + 1) * C],
                    wnat[:, i * C : (i + 1) * C],
                    ident[0:C, 0:C],
                )
            A_ps = ps.tile([C, C], F32, tag="B")
            nc.tensor.matmul(
                A_ps[:], wnat[:, 0 * C : 1 * C], wnat[:, 1 * C : 2 * C]
            )
            A_bf = constp.tile([C, C], BF16)
            nc.vector.tensor_copy(A_bf[:], A_ps[:])
            WpTf = constp.tile([C, C], F32)
            nc.vector.tensor_copy(WpTf[:], wT_ps[:, 3 * C : 4 * C])
            B_ps = ps.tile([C, C], F32, tag="Y")
            nc.tensor.matmul(B_ps[:], wnat[:, 2 * C : 3 * C], WpTf[:])

            # B_aug65 [C+1, C+1]: [Wv^T Wp^T, 0; bp, 1].  With the constant
            # ones row 64 of each xt tile (K=65 matmul), xB = x@B + bp and
            # col 64 = 1 (softmax sums) - no separate bias matmul.
            bp_row = constp.tile([1, C], F32)
            nc.sync.dma_start(bp_row[:], bp_ext.ap().unsqueeze(0))
            B_aug65 = constp.tile([C + 1, C + 1], BF16)
            nc.vector.tensor_copy(B_aug65[0:C, 0:C], B_ps[:])
            nc.vector.memset(B_aug65[0:C, C : C + 1], 0.0)
            nc.vector.tensor_copy(B_aug65[C : C + 1, 0:C], bp_row[:])
            nc.vector.memset(B_aug65[C : C + 1, C : C + 1], 1.0)

            # persistent xT tiles [C+1, TOK]: rows 0:64 rewritten per tile
            # (natural token order), row 64 = constant ones (bias fold)
            XT_BUFS = 6
            xt_tiles = [
                constp.tile(
                    [C + 1, TOK], BF16, tag=f"xt{i}", name=f"xt{i}"
                )
                for i in range(XT_BUFS)
            ]
            for i in range(XT_BUFS):
                nc.vector.memset(xt_tiles[i][C : C + 1, :], 1.0)

            # ---- software-pipelined main loop ----
            # One stage per cross-engine producer/consumer hop, one iteration
            # of skew each, emitted in REVERSE lag order so every consumer
            # precedes next tiles' producers in each engine's program order.
            state = {}

            def s_load(i):
                if i % 2 == 0:
                    st0 = i * TOK
                    x_sb2 = xin_pool.tile([128, 2, NCH * C], F32, tag="x_sb")
                    nc.sync.dma_start(
                        x_sb2[:],
                        x_flat[st0 : st0 + 2 * TOK, :].rearrange(
                            "(u p m) c -> p u (m c)", u=2, m=4
                        ),
                    )
                    state[i] = {"x2": x_sb2}
                    state[i + 1] = {"x2": x_sb2}

            def s_transpose(i):
                # f32 transposes to contiguous (m p)-packed psum (partition 0
                # only - a hardware rule for transposes), casting copy after
                x_sb = state[i]["x2"][:, i % 2, :]
                xT_ps = ps.tile([C, TOK], F32, tag="A")
                for m in range(NCH):
                    nc.tensor.transpose(
                        xT_ps[:, m * 128 : (m + 1) * 128],
                        x_sb[:, m * C : (m + 1) * C],
                        ident[:],
                    )
                state[i]["xT_ps"] = xT_ps

            def s_xt_cast(i):
                xp = xp_pool.tile([C, TOK], BF16, tag="xp")
                src = state[i].pop("xT_ps")
                nc.scalar.copy(xp[:], src[:])
                if "xt_cast" in dup:
                    nc.scalar.copy(xp[:], src[:])
                state[i]["xp"] = xp

            def s_unperm(i):
                xt = xt_tiles[i % XT_BUFS]
                for _ in range(2 if "unperm" in dup else 1):
                    nc.gpsimd.tensor_copy(
                        xt[0:C, :].rearrange("c (p m) -> c m p", m=4),
                        state[i]["xp"][:].rearrange("c (m p) -> c m p", p=128),
                    )
                state[i]["xT"] = xt

            def s_ht(i):
                # hT = A^T x^T on the packed xT (no need to wait for unperm)
                hT_ps = ps.tile([C, TOK], F32, tag="B")
                nc.tensor.matmul(hT_ps[:], A_bf[:], state[i]["xp"][:])
                state[i]["hT_ps"] = hT_ps

            def s_ht_copy(i):
                hp = hp_pool.tile([C, TOK], BF16, tag="hp")
                src = state[i].pop("hT_ps")
                nc.vector.tensor_copy(hp[:], src[:])
                if "ht_cast" in dup:
                    nc.vector.tensor_copy(hp[:], src[:])
                state[i]["hp"] = hp

            def s_ht_unperm(i):
                hT = ht_pool.tile([C, TOK], BF16, tag="hT")
                src = state[i].pop("hp")
                for _ in range(2 if "unperm" in dup else 1):
                    nc.gpsimd.tensor_copy(
                        hT[:].rearrange("c (p m) -> c m p", m=4),
                        src[:].rearrange("c (m p) -> c m p", p=128),
                    )
                state[i]["hT"] = hT

            def s_scores(i):
                xt = state[i]["xT"]
                hT = state[i]["hT"]
                weiT_ps = ps.tile([128, NCH, T], F32, tag="D")
                nc.tensor.matmul(
                    weiT_ps[:],
                    ident_bf[:],
                    maskS2[:].unsqueeze(1).broadcast_to([128, NCH, T]),
                    start=True, stop=False, skip_group_check=True,
                )
                for b in range(NB):
                    j, par = b // 2, b % 2
                    nc.tensor.matmul(
                        weiT_ps[par * T : (par + 1) * T, j, :],
                        xt[0:C, b * T : (b + 1) * T],
                        hT[:, b * T : (b + 1) * T],
                        start=False, stop=(b == NB - 1),
                        tile_position=(0, 64 * par),
                        skip_group_check=True,
                    )
                state[i]["weiT_ps"] = weiT_ps

            def s_exp(i):
                weiT_e = wei_pool.tile([128, NCH, T], BF16, tag="weiT_e")
                src = state[i].pop("weiT_ps")
                for _ in range(2 if "exp" in dup else 1):
                    nc.scalar.activation(
                        weiT_e[:], src[:],
                        mybir.ActivationFunctionType.Exp, scale=0.125,
                    )
                state[i]["wei"] = weiT_e

            def s_xb(i):
                xt = state[i]["xT"]
                xB_ps = ps.tile([128, NCH, C + 1], F32, tag="A")
                for j in range(NCH):
                    nc.tensor.matmul(
                        xB_ps[:, j, :],
                        xt[:, j * 128 : (j + 1) * 128],
                        B_aug65[:],
                    )
                state[i]["xB_ps"] = xB_ps

            def s_xb_copy(i):
                xB = xb_pool.tile([128, NCH, C + 1], BF16, tag="xB")
                src = state[i].pop("xB_ps")
                for _ in range(2 if "xb_copy" in dup else 1):
                    nc.vector.tensor_copy(xB[:], src[:])
                state[i]["xB"] = xB

            def s_y(i):
                weiT_e = state[i]["wei"]
                xB = state[i]["xB"]
                y_ps = ps.tile([128, NCH, C + 2], F32, tag="Y")
                for j in range(NCH):
                    nc.tensor.matmul(
                        y_ps[0:T, j, 0 : C + 1],
                        weiT_e[0:T, j, :], xB[0:T, j, :],
                    )
                    nc.tensor.matmul(
                        y_ps[T:128, j, 0 : C + 1],
                        weiT_e[T:128, j, :], xB[T:128, j, :],
                        tile_position=(64, 64),
                    )
                state[i]["y_ps"] = y_ps

            def s_fin(i):
                y_ps = state[i].pop("y_ps")
                recip = rc_pool.tile([128, NCH], F32, tag="recip")
                nc.vector.reciprocal(recip[:], y_ps[:, :, C : C + 1])
                if i % 2 == 0:
                    y2 = yout_pool.tile([128, 2, NCH, C], F32, tag="y_sb")
                    state[i]["y2"] = y2
                    if i + 1 in state:
                        state[i + 1]["y2"] = y2
                y_sb2 = state[i]["y2"]
                for _ in range(2 if "scale" in dup else 1):
                    nc.vector.tensor_tensor(
                        y_sb2[:, i % 2],
                        y_ps[:, :, 0:C],
                        recip[:].unsqueeze(2).broadcast_to([128, NCH, C]),
                        mybir.AluOpType.mult,
                    )
                t0 = i * TOK
                nc.sync.dma_start(
                    out_flat[t0 : t0 + TOK, :].rearrange(
                        "(j p) c -> p j c", p=128
                    ),
                    y_sb2[:, i % 2],
                )
                del state[i]

            stages = [
                (0, s_load),
                (2, s_transpose),
                (3, s_xt_cast),
                (4, s_unperm),
                (4, s_ht),
                (5, s_ht_copy),
                (6, s_ht_unperm),
                (7, s_scores),
                (8, s_exp),
                (9, s_xb),
                (10, s_xb_copy),
                (11, s_y),
                (12, s_fin),
            ]
            if n_stages_dbg is not None:
                stages = stages[:n_stages_dbg]
            max_lag = stages[-1][0]
            emit_order = sorted(stages, key=lambda s: -s[0])

            rep_ctx = tc.For_i(0, reps, 1) if reps > 1 else None
            if rep_ctx is not None:
                rep_ctx.__enter__()
            for it in range(n_tiles + max_lag):
                for lag, stage in emit_order:
                    i = it - lag
                    if 0 <= i < n_tiles:
                        stage(i)
            if rep_ctx is not None:
                rep_ctx.__exit__(None, None, None)

    nc.compile()
    return nc


_NC_CACHE = {}


def _get_nc(b_loc, batches_per_tile=8):
    key = (b_loc, batches_per_tile)
    if key not in _NC_CACHE:
        _NC_CACHE[key] = build_nc(b_loc, batches_per_tile)
    return _NC_CACHE[key]


def kernel(x, Wk, Wq, Wv, Wp, bp):
    from concourse.bass_utils import run_bass_kernel_spmd

    x = np.ascontiguousarray(x, dtype=np.float32)
    weights = {
        "Wk": np.ascontiguousarray(Wk, dtype=np.float32),
        "Wq": np.ascontiguousarray(Wq, dtype=np.float32),
        "Wv": np.ascontiguousarray(Wv, dtype=np.float32),
        "Wp": np.ascontiguousarray(Wp, dtype=np.float32),
        "bp": np.ascontiguousarray(bp, dtype=np.float32),
    }
    nc = _get_nc(B_LOC)
    in_maps = [
        {"x": x[i * B_LOC : (i + 1) * B_LOC], **weights} for i in range(N_CORES)
    ]
    res = run_bass_kernel_spmd(nc, in_maps, core_ids=list(range(N_CORES)))
    outs = [res.results[i]["out"] for i in range(N_CORES)]
    return np.concatenate(outs, axis=0)



# revision 23
# speedup vs baseline: 3.4024x; 3.4024x over previous
"""Causal attention (B=4096, T=64, C=64) on 8 TRN2 NeuronCores, pure data parallel.

Per core: x shard [512, 64, 64]. 512-token macro-tiles (8 batches). The x
load keeps 1KB DMA descriptors (partition p <- tokens 4p..4p+3, m-blocks),
and the resulting "packed" token order (block m, col p <-> token 4p+m) is
carried through the whole tile unchanged — no gpsimd un-permute. Batch-pair
slices are strided APs (pair j = cols 32j:32j+32 of each m-block), the
causal mask constant is pre-permuted, and the output DMA un-permutes via
its access pattern (256B descriptors, same as a natural layout would need).

Pipeline per tile (all matmul operands bf16, full 128-partition stationaries,
no tile_position anywhere):
  Pool: cast x f32->bf16.
  PE: 4 transposes [128,64]->[64,128] (bf16, 1 cyc/row) into one psum tile.
  DVE: single [64, 512] 2x copy psum->SBUF xt (rows 0:64 of a persistent
    [65, 4, 128] tile whose row 64 is constant ones - bias fold).
  PE: hT = A^T xT, A = Wq^T Wk (one N=512 matmul, bf16 psum out).
  DVE: 2x copy -> ht SBUF.
  PE: scores as 4 batch-PAIR matmuls: stationary xt[0:64, :, 32j:32j+32]
    (128 cols), stream ht same cols, N=128 -> weiT [128 s, j, 128 t] f32
    psum. Off-diagonal (cross-batch) quadrants get -inf from the mask-init
    matmul (ident stationary, streams the pre-permuted pair mask), so after
    exp they are exactly 0 and batch pairs can be CONSOLIDATED downstream.
  Act: exp(0.125 * weiT) -> bf16 wei_e (zeros in cross-batch quadrants).
  PE: xB = [x|1] @ B_aug65 per pair (B = Wv^T Wp^T, bias row bp; col 64 = 1
    sources the softmax sums), K=65, bf16 psum out.
  Act: copy -> xB SBUF.
  PE: y per pair: ONE K=128 matmul (stationary wei_e pair [128,128], stream
    xB pair [128, 65]) - cross-batch zeros make the consolidation exact.
  DVE: recip of sums col + broadcast multiply (y*recip + bp fold) -> f32 out.

Emission is software-pipelined (12 stages, 1 tile of skew each, reverse-lag
emission order); PSUM tags A/B/D/Y ring-buffer across iterations.
"""

import numpy as np
import ml_dtypes

import concourse.bass as bass
import concourse.mybir as mybir
import concourse.tile as tile
import concourse.masks as masks
from concourse import bacc

F32 = mybir.dt.float32
BF16 = mybir.dt.bfloat16

N_CORES = 8
B, T, C = 4096, 64, 64
B_LOC = B // N_CORES  # 512 batches per core

MASK_VAL = -1e9


def build_nc(b_loc=B_LOC, batches_per_tile=8, reps=1, n_stages_dbg=None, dup=()):
    """Build the single-core Bass graph (SPMD: same graph on all 8 cores)."""
    assert b_loc % batches_per_tile == 0
    n_tiles = b_loc // batches_per_tile
    TOK = batches_per_tile * T              # tokens per macro tile (512)
    NCH = TOK // 128                        # m-blocks / batch pairs per tile (4)

    nc = bacc.Bacc("TRN2", target_bir_lowering=False, debug=False)

    x_ext = nc.declare_dram_parameter("x", [b_loc, T, C], F32, isOutput=False)
    Wk_ext = nc.declare_dram_parameter("Wk", [C, C], F32, isOutput=False)
    Wq_ext = nc.declare_dram_parameter("Wq", [C, C], F32, isOutput=False)
    Wv_ext = nc.declare_dram_parameter("Wv", [C, C], F32, isOutput=False)
    Wp_ext = nc.declare_dram_parameter("Wp", [C, C], F32, isOutput=False)
    bp_ext = nc.declare_dram_parameter("bp", [C], F32, isOutput=False)
    out_ext = nc.declare_dram_parameter("out", [b_loc, T, C], F32, isOutput=True)

    x_flat = x_ext.ap().rearrange("b t c -> (b t) c")
    out_flat = out_ext.ap().rearrange("b t c -> (b t) c")

    bf = ml_dtypes.bfloat16
    # Pair mask [s-row in sigma order, t natural]. Keys land pair-contiguous
    # but sigma-packed (row 32m+p <-> rel token 4p+m); queries (t) are
    # natural. Causal within a batch, -inf across batches (zeros after exp
    # -> exact pair consolidation of the y matmuls).
    tau = 4 * (np.arange(128) % 32) + (np.arange(128) // 32)
    bat_s, pos_s = tau // T, tau % T
    bat_t, pos_t = np.arange(128) // T, np.arange(128) % T
    maskP = np.where(
        (bat_s[:, None] == bat_t[None, :]) & (pos_s[:, None] <= pos_t[None, :]),
        0.0,
        MASK_VAL * 8.0,
    ).astype(np.float32)
    maskP_dram = nc.inline_tensor(maskP.astype(bf), name="maskP_const")
    ident_dram = nc.inline_tensor(np.eye(128, dtype=np.float32), name="ident_const")

    with tile.TileContext(nc) as tc:
        with (
            tc.tile_pool(name="const", bufs=1) as constp,
            tc.tile_pool(name="xin", bufs=4) as xin_pool,
            tc.tile_pool(name="xbf", bufs=3) as xbf_pool,
            tc.tile_pool(name="ht", bufs=3) as ht_pool,
            tc.tile_pool(name="wei", bufs=4) as wei_pool,
            tc.tile_pool(name="xb", bufs=3) as xb_pool,
            tc.tile_pool(name="rc", bufs=3) as rc_pool,
            tc.tile_pool(name="yout", bufs=4) as yout_pool,
            tc.tile_pool(name="ps", bufs=2, space="PSUM") as ps,
        ):
            # ---- one-time constants ----
            ident = constp.tile([128, 128], F32)
            nc.sync.dma_start(ident[:], ident_dram.ap())
            ident_bf = constp.tile([128, 128], BF16)
            masks.make_identity(nc, ident_bf[:])
            maskPt = constp.tile([128, 128], BF16)
            nc.sync.dma_start(maskPt[:], maskP_dram.ap())

            # weights: natural DMA (contiguous); A = Wq^T Wk, B = Wv^T Wp^T
            wnat = constp.tile([C, 4 * C], F32)
            for i, w_ext in enumerate((Wq_ext, Wk_ext, Wv_ext, Wp_ext)):
                nc.sync.dma_start(wnat[:, i * C : (i + 1) * C], w_ext.ap())
            wT_ps = ps.tile([C, C], F32, tag="A")
            nc.tensor.transpose(
                wT_ps[:], wnat[:, 3 * C : 4 * C], ident[0:C, 0:C]
            )
            WpTf = constp.tile([C, C], F32)
            nc.vector.tensor_copy(WpTf[:], wT_ps[:])
            A_ps = ps.tile([C, C], F32, tag="B")
            nc.tensor.matmul(
                A_ps[:], wnat[:, 0 * C : 1 * C], wnat[:, 1 * C : 2 * C]
            )
            A_bf = constp.tile([C, C], BF16)
            nc.vector.tensor_copy(A_bf[:], A_ps[:])
            B_ps = ps.tile([C, C], F32, tag="Y")
            nc.tensor.matmul(B_ps[:], wnat[:, 2 * C : 3 * C], WpTf[:])

            # B_aug65 [C+1, C+1]: [Wv^T Wp^T, 0; bp, 1].  With the constant
            # ones row 64 of each xt tile (K=65 matmul), xB = x@B + bp and
            # col 64 = 1 (softmax sums source).
            bp_row = constp.tile([1, C], F32)
            nc.sync.dma_start(bp_row[:], bp_ext.ap().unsqueeze(0))
            B_aug65 = constp.tile([C + 1, C + 1], BF16)
            nc.vector.tensor_copy(B_aug65[0:C, 0:C], B_ps[:])
            nc.vector.memset(B_aug65[0:C, C : C + 1], 0.0)
            nc.vector.tensor_copy(B_aug65[C : C + 1, 0:C], bp_row[:])
            nc.vector.memset(B_aug65[C : C + 1, C : C + 1], 1.0)

            # persistent xt tiles [C+1, TOK]: rows 0:64 rewritten per tile
            # (natural token order), row 64 = constant ones (bias fold)
            XT_BUFS = 6
            xt_tiles = [
                constp.tile([C + 1, TOK], BF16, tag=f"xt{i}", name=f"xt{i}")
                for i in range(XT_BUFS)
            ]
            for i in range(XT_BUFS):
                nc.vector.memset(xt_tiles[i][C : C + 1, :], 1.0)

            # ---- software-pipelined main loop ----
            state = {}

            def s_load(i):
                if i % 2 == 0:
                    st0 = i * TOK
                    x_sb2 = xin_pool.tile([128, 2, NCH * C], F32, tag="x_sb")
                    nc.sync.dma_start(
                        x_sb2[:],
                        x_flat[st0 : st0 + 2 * TOK, :].rearrange(
                            "(u p m) c -> p u (m c)", u=2, m=4
                        ),
                    )
                    state[i] = {"x2": x_sb2}
                    state[i + 1] = {"x2": x_sb2}

            def s_cast(i):
                xbf = xbf_pool.tile([128, NCH * C], BF16, tag="xbf")
                for _ in range(2 if "cast" in dup else 1):
                    nc.gpsimd.tensor_copy(xbf[:], state[i]["x2"][:, i % 2, :])
                state[i]["xbf"] = xbf

            def s_transpose(i):
                # Transpose m-block m holds tokens {4p+m}. Column p goes to
                # pair p//32 (free region 128*(p//32)) at offset 32m + p%32:
                # pairs end up CONTIGUOUS (sigma-packed within the pair),
                # and every chunk offset is 64B-aligned in psum.
                xbf = state[i]["xbf"]
                xT_ps = ps.tile([C, TOK], BF16, tag="A")
                xT_v = xT_ps[:].rearrange("c (j m p) -> c m j p", j=4, m=4)
                for m in range(NCH):
                    nc.tensor.transpose(
                        xT_v[:, m], xbf[:, m * C : (m + 1) * C], ident_bf[:]
                    )
                state[i]["xT_ps"] = xT_ps

            def s_xt_copy(i):
                # natural psum -> natural SBUF (2x mode, both bf16)
                xt = xt_tiles[i % XT_BUFS]
                src = state[i].pop("xT_ps")
                for _ in range(2 if "xt_copy" in dup else 1):
                    nc.vector.tensor_copy(xt[0:C, :], src[:])
                state[i]["xt"] = xt

            def s_ht(i):
                hT_ps = ps.tile([C, TOK], F32, tag="B")
                nc.tensor.matmul(
                    hT_ps[:], A_bf[:], state[i]["xt"][0:C, :]
                )
                state[i]["hT_ps"] = hT_ps

            def s_ht_copy(i):
                ht = ht_pool.tile([C, TOK], BF16, tag="ht")
                src = state[i].pop("hT_ps")
                for _ in range(2 if "ht_copy" in dup else 1):
                    nc.scalar.copy(ht[:], src[:])
                state[i]["ht"] = ht

            def s_scores(i):
                xt, ht = state[i]["xt"], state[i]["ht"]
                weiT_ps = ps.tile([128, NCH, 128], F32, tag="D")
                nc.tensor.matmul(
                    weiT_ps[:],
                    ident_bf[:],
                    maskPt[:].unsqueeze(1).broadcast_to([128, NCH, 128]),
                    start=True, stop=False, skip_group_check=True,
                )
                ht_nat = ht[0:C, :].rearrange("c (j m p) -> c j p m", j=4, m=4)
                for j in range(NCH):
                    nc.tensor.matmul(
                        weiT_ps[:, j, :],
                        xt[0:C, 128 * j : 128 * (j + 1)],
                        ht_nat[:, j],
                        start=False, stop=(j == NCH - 1),
                        skip_group_check=True,
                    )
                state[i]["weiT_ps"] = weiT_ps

            def s_exp(i):
                wei_e = wei_pool.tile([128, NCH, 128], BF16, tag="wei")
                src = state[i].pop("weiT_ps")
                for _ in range(2 if "exp" in dup else 1):
                    nc.scalar.activation(
                        wei_e[:], src[:],
                        mybir.ActivationFunctionType.Exp, scale=0.125,
                    )
                state[i]["wei"] = wei_e

            def s_xb(i):
                xt = state[i]["xt"]
                xB_ps = ps.tile([128, NCH, C + 1], F32, tag="A")
                for j in range(NCH):
                    nc.tensor.matmul(
                        xB_ps[:, j, :],
                        xt[:, 128 * j : 128 * (j + 1)],
                        B_aug65[:],
                    )
                state[i]["xB_ps"] = xB_ps

            def s_xb_copy(i):
                xB = xb_pool.tile([128, NCH, C + 1], BF16, tag="xB")
                src = state[i].pop("xB_ps")
                for _ in range(2 if "xb_copy" in dup else 1):
                    nc.vector.tensor_copy(xB[:], src[:])
                state[i]["xB"] = xB

            def s_y(i):
                wei_e, xB = state[i].pop("wei"), state[i].pop("xB")
                y_ps = ps.tile([128, NCH, C + 1], F32, tag="Y")
                for j in range(NCH):
                    nc.tensor.matmul(
                        y_ps[:, j, :], wei_e[:, j, :], xB[:, j, :]
                    )
                state[i]["y_ps"] = y_ps

            def s_fin(i):
                y_ps = state[i].pop("y_ps")
                recip = rc_pool.tile([128, NCH], F32, tag="recip")
                nc.vector.reciprocal(recip[:], y_ps[:, :, C : C + 1])
                if i % 2 == 0:
                    y2 = yout_pool.tile([128, 2, NCH, C], F32, tag="y_sb")
                    state[i]["y2"] = y2
                    if i + 1 in state:
                        state[i + 1]["y2"] = y2
                y_sb2 = state[i]["y2"]
                for _ in range(2 if "scale" in dup else 1):
                    nc.vector.tensor_tensor(
                        y_sb2[:, i % 2],
                        y_ps[:, :, 0:C],
                        recip[:].unsqueeze(2).broadcast_to([128, NCH, C]),
                        mybir.AluOpType.mult,
                    )
                t0 = i * TOK
                nc.sync.dma_start(
                    out_flat[t0 : t0 + TOK, :].rearrange(
                        "(j q) c -> q j c", j=NCH
                    ),
                    y_sb2[:, i % 2],
                )
                del state[i]

            stages = [
                (0, s_load),
                (2, s_cast),
                (3, s_transpose),
                (4, s_xt_copy),
                (5, s_ht),
                (6, s_ht_copy),
                (7, s_scores),
                (8, s_exp),
                (9, s_xb),
                (10, s_xb_copy),
                (11, s_y),
                (12, s_fin),
            ]
            if n_stages_dbg is not None:
                stages = stages[:n_stages_dbg]
            max_lag = stages[-1][0]
            emit_order = sorted(stages, key=lambda s: -s[0])

            rep_ctx = tc.For_i(0, reps, 1) if reps > 1 else None
            if rep_ctx is not None:
                rep_ctx.__enter__()
            for it in range(n_tiles + max_lag):
                for lag, stage in emit_order:
                    i = it - lag
                    if 0 <= i < n_tiles:
                        stage(i)
            if rep_ctx is not None:
                rep_ctx.__exit__(None, None, None)

    nc.compile()
    return nc


_NC_CACHE = {}


def _get_nc(b_loc, batches_per_tile=8):
    key = (b_loc, batches_per_tile)
    if key not in _NC_CACHE:
        _NC_CACHE[key] = build_nc(b_loc, batches_per_tile)
    return _NC_CACHE[key]


def kernel(x, Wk, Wq, Wv, Wp, bp):
    from concourse.bass_utils import run_bass_kernel_spmd

    x = np.ascontiguousarray(x, dtype=np.float32)
    weights = {
        "Wk": np.ascontiguousarray(Wk, dtype=np.float32),
        "Wq": np.ascontiguousarray(Wq, dtype=np.float32),
        "Wv": np.ascontiguousarray(Wv, dtype=np.float32),
        "Wp": np.ascontiguousarray(Wp, dtype=np.float32),
        "bp": np.ascontiguousarray(bp, dtype=np.float32),
    }
    nc = _get_nc(B_LOC)
    in_maps = [
        {"x": x[i * B_LOC : (i + 1) * B_LOC], **weights} for i in range(N_CORES)
    ]
    res = run_bass_kernel_spmd(nc, in_maps, core_ids=list(range(N_CORES)))
    outs = [res.results[i]["out"] for i in range(N_CORES)]
    return np.concatenate(outs, axis=0)


# revision 37
# speedup vs baseline: 6.8940x; 2.0262x over previous
"""Causal attention (B=4096, T=64, C=64) on 8 TRN2 NeuronCores, pure data parallel.

Per core: x shard [512, 64, 64]. 512-token macro-tiles (8 batches). The x
load keeps 1KB DMA descriptors (partition p <- tokens 4p..4p+3, m-blocks),
and the resulting "packed" token order (block m, col p <-> token 4p+m) is
carried through the whole tile unchanged — no gpsimd un-permute. Batch-pair
slices are strided APs (pair j = cols 32j:32j+32 of each m-block), the
causal mask constant is pre-permuted, and the output DMA un-permutes via
its access pattern (256B descriptors, same as a natural layout would need).

Pipeline per tile (all matmul operands bf16, full 128-partition stationaries,
no tile_position anywhere):
  Pool: cast x f32->bf16.
  PE: 4 transposes [128,64]->[64,128] (bf16, 1 cyc/row) into one psum tile.
  DVE: single [64, 512] 2x copy psum->SBUF xt (rows 0:64 of a persistent
    [65, 4, 128] tile whose row 64 is constant ones - bias fold).
  PE: hT = A^T xT, A = Wq^T Wk (one N=512 matmul, bf16 psum out).
  DVE: 2x copy -> ht SBUF.
  PE: scores as 4 batch-PAIR matmuls: stationary xt[0:64, :, 32j:32j+32]
    (128 cols), stream ht same cols, N=128 -> weiT [128 s, j, 128 t] f32
    psum. Off-diagonal (cross-batch) quadrants get -inf from the mask-init
    matmul (ident stationary, streams the pre-permuted pair mask), so after
    exp they are exactly 0 and batch pairs can be CONSOLIDATED downstream.
  Act: exp(0.125 * weiT) -> bf16 wei_e (zeros in cross-batch quadrants).
  PE: xB = [x|1] @ B_aug65 per pair (B = Wv^T Wp^T, bias row bp; col 64 = 1
    sources the softmax sums), K=65, bf16 psum out.
  Act: copy -> xB SBUF.
  PE: y per pair: ONE K=128 matmul (stationary wei_e pair [128,128], stream
    xB pair [128, 65]) - cross-batch zeros make the consolidation exact.
  DVE: recip of sums col + broadcast multiply (y*recip + bp fold) -> f32 out.

Emission is software-pipelined (12 stages, 1 tile of skew each, reverse-lag
emission order); PSUM tags A/B/D/Y ring-buffer across iterations.
"""

import numpy as np
import ml_dtypes

import concourse.bass as bass
import concourse.mybir as mybir
import concourse.tile as tile
import concourse.masks as masks
from concourse import bacc

F32 = mybir.dt.float32
BF16 = mybir.dt.bfloat16

N_CORES = 8
B, T, C = 4096, 64, 64
B_LOC = B // N_CORES  # 512 batches per core

MASK_VAL = -1e9


def build_nc(b_loc=B_LOC, batches_per_tile=8, reps=1, n_stages_dbg=None, dup=()):
    """Build the single-core Bass graph (SPMD: same graph on all 8 cores)."""
    assert b_loc % batches_per_tile == 0
    n_tiles = b_loc // batches_per_tile
    TOK = batches_per_tile * T              # tokens per macro tile (512)
    NCH = TOK // 128                        # m-blocks / batch pairs per tile (4)

    nc = bacc.Bacc("TRN2", target_bir_lowering=False, debug=False)

    x_ext = nc.declare_dram_parameter("x", [b_loc, T, C], F32, isOutput=False)
    Wk_ext = nc.declare_dram_parameter("Wk", [C, C], F32, isOutput=False)
    Wq_ext = nc.declare_dram_parameter("Wq", [C, C], F32, isOutput=False)
    Wv_ext = nc.declare_dram_parameter("Wv", [C, C], F32, isOutput=False)
    Wp_ext = nc.declare_dram_parameter("Wp", [C, C], F32, isOutput=False)
    bp_ext = nc.declare_dram_parameter("bp", [C], F32, isOutput=False)
    out_ext = nc.declare_dram_parameter("out", [b_loc, T, C], F32, isOutput=True)

    x_flat = x_ext.ap().rearrange("b t c -> (b t) c")
    out_flat = out_ext.ap().rearrange("b t c -> (b t) c")

    bf = ml_dtypes.bfloat16
    # Pair mask [s-row in sigma order, t natural]. Keys land pair-contiguous
    # but sigma-packed (row 32m+p <-> rel token 4p+m); queries (t) are
    # natural. Causal within a batch, -inf across batches (zeros after exp
    # -> exact pair consolidation of the y matmuls).
    tau = 4 * (np.arange(128) % 32) + (np.arange(128) // 32)
    bat_s, pos_s = tau // T, tau % T
    bat_t, pos_t = np.arange(128) // T, np.arange(128) % T
    maskP = np.where(
        (bat_s[:, None] == bat_t[None, :]) & (pos_s[:, None] <= pos_t[None, :]),
        0.0,
        MASK_VAL * 8.0,
    ).astype(np.float32)
    maskP_dram = nc.inline_tensor(maskP.astype(bf), name="maskP_const")
    ident_dram = nc.inline_tensor(np.eye(128, dtype=np.float32), name="ident_const")

    with tile.TileContext(nc) as tc:
        with (
            tc.tile_pool(name="const", bufs=1) as constp,
            tc.tile_pool(name="xin", bufs=4) as xin_pool,
            tc.tile_pool(name="xbf", bufs=4) as xbf_pool,
            tc.tile_pool(name="ht", bufs=4) as ht_pool,
            tc.tile_pool(name="wei", bufs=5) as wei_pool,
            tc.tile_pool(name="xb", bufs=4) as xb_pool,
            tc.tile_pool(name="rc", bufs=4) as rc_pool,
            tc.tile_pool(name="yout", bufs=4) as yout_pool,
            tc.tile_pool(name="ps", bufs=2, space="PSUM") as ps,
        ):
            # ---- one-time constants ----
            ident = constp.tile([128, 128], F32)
            nc.sync.dma_start(ident[:], ident_dram.ap())
            ident_bf = constp.tile([128, 128], BF16)
            masks.make_identity(nc, ident_bf[:])
            maskPt = constp.tile([128, 128], BF16)
            nc.sync.dma_start(maskPt[:], maskP_dram.ap())

            # weights: natural DMA (contiguous); A = Wq^T Wk, B = Wv^T Wp^T
            wnat = constp.tile([C, 4 * C], F32)
            for i, w_ext in enumerate((Wq_ext, Wk_ext, Wv_ext, Wp_ext)):
                nc.sync.dma_start(wnat[:, i * C : (i + 1) * C], w_ext.ap())
            wT_ps = ps.tile([C, C], F32, tag="A")
            nc.tensor.transpose(
                wT_ps[:], wnat[:, 3 * C : 4 * C], ident[0:C, 0:C]
            )
            WpTf = constp.tile([C, C], F32)
            nc.vector.tensor_copy(WpTf[:], wT_ps[:])
            A_ps = ps.tile([C, C], F32, tag="B")
            nc.tensor.matmul(
                A_ps[:], wnat[:, 0 * C : 1 * C], wnat[:, 1 * C : 2 * C]
            )
            A_bf = constp.tile([C, C], BF16)
            nc.vector.tensor_copy(A_bf[:], A_ps[:])
            B_ps = ps.tile([C, C], F32, tag="Y")
            nc.tensor.matmul(B_ps[:], wnat[:, 2 * C : 3 * C], WpTf[:])

            # B_aug65 [C+1, C+1]: [Wv^T Wp^T, 0; bp, 1].  With the constant
            # ones row 64 of each xt tile (K=65 matmul), xB = x@B + bp and
            # col 64 = 1 (softmax sums source).
            bp_row = constp.tile([1, C], F32)
            nc.sync.dma_start(bp_row[:], bp_ext.ap().unsqueeze(0))
            B_aug65 = constp.tile([C + 1, C + 1], BF16)
            nc.vector.tensor_copy(B_aug65[0:C, 0:C], B_ps[:])
            nc.vector.memset(B_aug65[0:C, C : C + 1], 0.0)
            nc.vector.tensor_copy(B_aug65[C : C + 1, 0:C], bp_row[:])
            nc.vector.memset(B_aug65[C : C + 1, C : C + 1], 1.0)

            # persistent xt tiles [C+1, TOK]: rows 0:64 rewritten per tile
            # (natural token order), row 64 = constant ones (bias fold)
            XT_BUFS = 9
            xt_tiles = [
                constp.tile([C + 1, TOK], BF16, tag=f"xt{i}", name=f"xt{i}")
                for i in range(XT_BUFS)
            ]
            for i in range(XT_BUFS):
                nc.vector.memset(xt_tiles[i][C : C + 1, :], 1.0)

            # ---- software-pipelined main loop ----
            state = {}

            def s_load(i):
                if i % 2 == 0:
                    st0 = i * TOK
                    x_sb2 = xin_pool.tile([128, 2, NCH * C], F32, tag="x_sb")
                    nc.sync.dma_start(
                        x_sb2[:],
                        x_flat[st0 : st0 + 2 * TOK, :].rearrange(
                            "(u p m) c -> p u (m c)", u=2, m=4
                        ),
                    )
                    state[i] = {"x2": x_sb2}
                    state[i + 1] = {"x2": x_sb2}

            def s_cast(i):
                xbf = xbf_pool.tile([128, NCH * C], BF16, tag="xbf")
                for _ in range(2 if "cast" in dup else 1):
                    nc.gpsimd.tensor_copy(xbf[:], state[i]["x2"][:, i % 2, :])
                state[i]["xbf"] = xbf

            def s_transpose(i):
                # Transpose m-block m holds tokens {4p+m}. Column p goes to
                # pair p//32 (free region 128*(p//32)) at offset 32m + p%32:
                # pairs end up CONTIGUOUS (sigma-packed within the pair),
                # and every chunk offset is 64B-aligned in psum.
                xbf = state[i]["xbf"]
                xT_ps = ps.tile([C, TOK], BF16, tag="A")
                xT_v = xT_ps[:].rearrange("c (j m p) -> c m j p", j=4, m=4)
                for m in range(NCH):
                    nc.tensor.transpose(
                        xT_v[:, m], xbf[:, m * C : (m + 1) * C], ident_bf[:]
                    )
                state[i]["xT_ps"] = xT_ps

            def s_xt_copy(i):
                # natural psum -> natural SBUF (2x mode, both bf16)
                xt = xt_tiles[i % XT_BUFS]
                src = state[i].pop("xT_ps")
                for _ in range(2 if "xt_copy" in dup else 1):
                    nc.vector.tensor_copy(xt[0:C, :], src[:])
                state[i]["xt"] = xt

            def s_ht(i):
                hT_ps = ps.tile([C, TOK], F32, tag="B")
                nc.tensor.matmul(
                    hT_ps[:], A_bf[:], state[i]["xt"][0:C, :]
                )
                state[i]["hT_ps"] = hT_ps

            def s_ht_copy(i):
                ht = ht_pool.tile([C, TOK], BF16, tag="ht")
                src = state[i].pop("hT_ps")
                for _ in range(2 if "ht_copy" in dup else 1):
                    nc.scalar.copy(ht[:], src[:])
                state[i]["ht"] = ht

            def s_scores(i):
                xt, ht = state[i]["xt"], state[i]["ht"]
                weiT_ps = ps.tile([128, NCH, 128], F32, tag="D")
                nc.tensor.matmul(
                    weiT_ps[:],
                    ident_bf[:],
                    maskPt[:].unsqueeze(1).broadcast_to([128, NCH, 128]),
                    start=True, stop=False, skip_group_check=True,
                )
                ht_nat = ht[0:C, :].rearrange("c (j m p) -> c j p m", j=4, m=4)
                for j in range(NCH):
                    nc.tensor.matmul(
                        weiT_ps[:, j, :],
                        xt[0:C, 128 * j : 128 * (j + 1)],
                        ht_nat[:, j],
                        start=False, stop=(j == NCH - 1),
                        skip_group_check=True,
                    )
                state[i]["weiT_ps"] = weiT_ps

            def s_exp(i):
                wei_e = wei_pool.tile([128, NCH, 128], BF16, tag="wei")
                src = state[i].pop("weiT_ps")
                for _ in range(2 if "exp" in dup else 1):
                    nc.scalar.activation(
                        wei_e[:], src[:],
                        mybir.ActivationFunctionType.Exp, scale=0.125,
                    )
                state[i]["wei"] = wei_e

            def s_xb(i):
                xt = state[i]["xt"]
                xB_ps = ps.tile([128, NCH, C + 1], F32, tag="A")
                for j in range(NCH):
                    nc.tensor.matmul(
                        xB_ps[:, j, :],
                        xt[:, 128 * j : 128 * (j + 1)],
                        B_aug65[:],
                    )
                state[i]["xB_ps"] = xB_ps

            def s_xb_copy(i):
                xB = xb_pool.tile([128, NCH, C + 1], BF16, tag="xB")
                src = state[i].pop("xB_ps")
                for _ in range(2 if "xb_copy" in dup else 1):
                    nc.vector.tensor_copy(xB[:], src[:])
                state[i]["xB"] = xB

            def s_y(i):
                wei_e, xB = state[i].pop("wei"), state[i].pop("xB")
                y_ps = ps.tile([128, NCH, C + 1], F32, tag="Y")
                for j in range(NCH):
                    nc.tensor.matmul(
                        y_ps[:, j, :], wei_e[:, j, :], xB[:, j, :]
                    )
                state[i]["y_ps"] = y_ps

            def s_fin(i):
                y_ps = state[i].pop("y_ps")
                recip = rc_pool.tile([128, NCH], F32, tag="recip")
                nc.vector.reciprocal(recip[:], y_ps[:, :, C : C + 1])
                if i % 2 == 0:
                    y2 = yout_pool.tile([128, 2, NCH, C], F32, tag="y_sb")
                    state[i]["y2"] = y2
                    if i + 1 in state:
                        state[i + 1]["y2"] = y2
                y_sb2 = state[i]["y2"]
                for _ in range(2 if "scale" in dup else 1):
                    nc.vector.tensor_tensor(
                        y_sb2[:, i % 2],
                        y_ps[:, :, 0:C],
                        recip[:].unsqueeze(2).broadcast_to([128, NCH, C]),
                        mybir.AluOpType.mult,
                    )
                if i % 2 == 1:
                    # one out-DMA per 2 tiles halves HWDGE/SP dispatch cost
                    t0 = (i - 1) * TOK
                    nc.sync.dma_start(
                        out_flat[t0 : t0 + 2 * TOK, :].rearrange(
                            "(u j q) c -> q u j c", u=2, j=NCH
                        ),
                        y_sb2[:],
                    )
                del state[i]

            stages = [
                (0, s_load),
                (2, s_cast),
                (3, s_transpose),
                (4, s_xt_copy),
                (6, s_ht),
                (7, s_ht_copy),
                (9, s_scores),
                (10, s_exp),
                (11, s_xb),
                (12, s_xb_copy),
                (13, s_y),
                (14, s_fin),
            ]
            if n_stages_dbg is not None:
                stages = stages[:n_stages_dbg]
            max_lag = stages[-1][0]
            emit_order = sorted(stages, key=lambda s: -s[0])

            rep_ctx = tc.For_i(0, reps, 1) if reps > 1 else None
            if rep_ctx is not None:
                rep_ctx.__enter__()
            for it in range(n_tiles + max_lag):
                for lag, stage in emit_order:
                    i = it - lag
                    if 0 <= i < n_tiles:
                        stage(i)
            if rep_ctx is not None:
                rep_ctx.__exit__(None, None, None)

    nc.compile()
    return nc


_NC_CACHE = {}


def _get_nc(b_loc, batches_per_tile=8):
    key = (b_loc, batches_per_tile)
    if key not in _NC_CACHE:
        _NC_CACHE[key] = build_nc(b_loc, batches_per_tile)
    return _NC_CACHE[key]


def kernel(x, Wk, Wq, Wv, Wp, bp):
    from concourse.bass_utils import run_bass_kernel_spmd

    x = np.ascontiguousarray(x, dtype=np.float32)
    weights = {
        "Wk": np.ascontiguousarray(Wk, dtype=np.float32),
        "Wq": np.ascontiguousarray(Wq, dtype=np.float32),
        "Wv": np.ascontiguousarray(Wv, dtype=np.float32),
        "Wp": np.ascontiguousarray(Wp, dtype=np.float32),
        "bp": np.ascontiguousarray(bp, dtype=np.float32),
    }
    nc = _get_nc(B_LOC)
    in_maps = [
        {"x": x[i * B_LOC : (i + 1) * B_LOC], **weights} for i in range(N_CORES)
    ]
    res = run_bass_kernel_spmd(nc, in_maps, core_ids=list(range(N_CORES)))
    outs = [res.results[i]["out"] for i in range(N_CORES)]
    return np.concatenate(outs, axis=0)
